# revision 32
# baseline (speedup 1.0000x reference)
"""Pre-LN multi-head attention block on 8 Trainium2 NeuronCores (Bass/Tile).

Reference computation (shapes hardcoded):
    qh = LN(q + qpos) @ Wq ; kh = LN(k + kpos) @ Wk ; vh = LN(v) @ Wv
    out = softmax(qh kh^T / 8) vh @ Wp + bp          (B=2, N=2048, D=1024, H=16)

Sharding (no collectives): 8 cores = (batch b, head-half hh, q-half qh).
Each core computes 8 heads x 1024 q-rows against all 2048 keys and a
partial output projection; the host sums the two head-half partials.

Device algorithm per core (all matmuls bf16 with f32 PSUM accumulation):
  - Host pre-transposes activations (layout only): x^T [1024, rows] f32.
  - LN is folded into the projections: mean-subtraction via an augmented
    1025th weight row (-colsum(W)) against a mean-row appended to x, and
    the rstd multiply rides the PSUM->SBUF copy pass. Stats (sum, sum of
    squares) come from ones-matmuls on the tensor engine; rstd via
    exp(-0.5*ln(var+eps)) on the scalar engine (Rsqrt table is banned).
  - Attention: S^T[keys, q] = khT-block.T @ qhT on PE; P^T = exp(S^T) on
    the scalar engine (no max subtraction needed: |S| < ~3); O^T = V.T@P^T
    with an all-ones column prepended to V so softmax row-sums accumulate
    in PSUM partition 0 of the same matmul; normalize by reciprocal.
  - Output projection from attnout^T chunks; host adds the two partials
    plus the bias epilogue (bv@Wv)@Wp + bp (exact: softmax rows sum to 1;
    k-side bias is softmax-invariant and dropped; q-side bias added on
    device only if nonzero).
"""
import os
import numpy as np
import ml_dtypes

from contextlib import ExitStack
from concourse import bass, bacc, tile, mybir
from concourse.bass_utils import run_bass_kernel_spmd

F32 = mybir.dt.float32
BF16 = mybir.dt.bfloat16
FP8 = mybir.dt.float8e4
AF = mybir.ActivationFunctionType
OP = mybir.AluOpType
DR = mybir.MatmulPerfMode.DoubleRow

B, NQ, NK, D, H = 2, 2048, 2048, 1024, 16
HD = D // H
SCALE = float(HD) ** -0.5
EPS = 1e-5

NCORE = 8
DOUT = 512          # per-core projection width (8 heads)
NQC = 1024          # per-core q rows
NHC = DOUT // HD    # 8 heads per core
NBLK_Q = NQC // 512
NBLK_K = NK // 512

# exec_time_ns of the last run when tracing is enabled (read by test.py)
LAST_RESULT = {}


def _build_graph(has_bqw: bool):
    nc = bacc.Bacc("TRN2", target_bir_lowering=False, debug=False,
                   num_devices=NCORE)

    d_qT = nc.dram_tensor("qT", [D, NQC], F32, kind="ExternalInput").ap()
    d_qposT = nc.dram_tensor("qposT", [D, NQC], F32, kind="ExternalInput").ap()
    d_kT = nc.dram_tensor("kT", [D, NK], F32, kind="ExternalInput").ap()
    d_kposT = nc.dram_tensor("kposT", [D, NK], F32, kind="ExternalInput").ap()
    d_vT = nc.dram_tensor("vT", [D, NK], F32, kind="ExternalInput").ap()
    d_wq = nc.dram_tensor("wq", [D + 1, DOUT], BF16, kind="ExternalInput").ap()
    d_wk = nc.dram_tensor("wk", [D + 1, DOUT], BF16, kind="ExternalInput").ap()
    d_wv = nc.dram_tensor("wv", [D + 1, DOUT], BF16, kind="ExternalInput").ap()
    d_wp = nc.dram_tensor("wp", [DOUT, D], BF16, kind="ExternalInput").ap()
    d_bqw = (nc.dram_tensor("bqw", [4, 128], F32, kind="ExternalInput").ap()
             if has_bqw else None)
    d_out = nc.dram_tensor("out", [NQC, D], F32, kind="ExternalOutput").ap()

    with tile.TileContext(nc) as tc, ExitStack() as es:
        persist = es.enter_context(tc.tile_pool(name="persist", bufs=1))

        # ---- persistent SBUF tensors ------------------------------------
        wq_t = persist.tile([128, 8, DOUT], BF16)
        wk_t = persist.tile([128, 8, DOUT], BF16)
        wv_t = persist.tile([128, 8, DOUT], BF16)
        wq_l = persist.tile([1, DOUT], BF16)
        wk_l = persist.tile([1, DOUT], BF16)
        wv_l = persist.tile([1, DOUT], BF16)
        wp_t = persist.tile([128, 4, D], BF16)
        ones_t = persist.tile([128, 128], BF16)
        ones8 = persist.tile([128, 2, 128], FP8)
        eps_t = persist.tile([128, 1], F32)
        qhT = persist.tile([128, 4, NQC], BF16)            # [512 dout, 1024 q]
        khT = persist.tile([128, 4, NK], BF16)             # [512 dout, 2048 k]
        vh = persist.tile([128, NBLK_K * 4, NHC * 65], BF16)  # per 128-key subblock
        # attnout^T, one tile per q-tile so the qt1 epilogue DMAs never
        # create a (tile-granular) false dep on qt0's output projection
        aout0 = persist.tile([128, 4, 512], BF16)
        aout1 = persist.tile([128, 4, 512], BF16)
        aout_t = [aout0, aout1]
        bqw_t = persist.tile([128, 4], F32) if has_bqw else None

        def load_weights():
            # dedicated (otherwise idle) gpsimd ring so weights never queue
            # behind activation blocks; wv first (v0 projects first)
            for w_t, w_l, d_w in ((wv_t, wv_l, d_wv), (wq_t, wq_l, d_wq),
                                  (wk_t, wk_l, d_wk)):
                nc.gpsimd.dma_start(w_t[:], d_w[0:D, :].rearrange("(c p) n -> p c n", p=128))
                nc.gpsimd.dma_start(w_l[:], d_w[D:D + 1, :])
            if has_bqw:
                nc.gpsimd.dma_start(bqw_t[:], d_bqw.rearrange("d p -> p d"))

        def load_wp():
            # wp isn't needed until phase C; deferring it keeps early HBM
            # bandwidth for the activation blocks
            nc.gpsimd.dma_start(wp_t[:], d_wp.rearrange("(c p) n -> p c n", p=128))

        nc.vector.memset(ones_t[:], 1.0)
        nc.vector.memset(ones8[:], 1.0)
        nc.vector.memset(eps_t[:], EPS)
        # all-ones column at the tail of each 65-wide V group
        nc.vector.memset(vh[:].rearrange("p s (h u) -> p s h u", u=65)[:, :, :, 64:65], 1.0)

        # ---- pools (all live for the whole kernel; PSUM budget = 8 banks:
        #      proj 1 + O 1 + stats 2 + S 2x2 = 8) --------------------------
        pools = es.enter_context(ExitStack())
        xin_p = pools.enter_context(tc.tile_pool(name="xin", bufs=2))
        xpos_p = pools.enter_context(tc.tile_pool(name="xpos", bufs=2))
        xbf_p = pools.enter_context(tc.tile_pool(name="xbf", bufs=2))
        xsq_p = pools.enter_context(tc.tile_pool(name="xsq", bufs=1))
        sm_p = pools.enter_context(tc.tile_pool(name="sm", bufs=2))
        rrep_p = pools.enter_context(tc.tile_pool(name="rrepp", bufs=2))
        mrow_p = pools.enter_context(tc.tile_pool(name="mrowp", bufs=2))
        rv_p = pools.enter_context(tc.tile_pool(name="rvp", bufs=2))
        p_sb = pools.enter_context(tc.tile_pool(name="psb", bufs=2))
        ep_sb = pools.enter_context(tc.tile_pool(name="epsb", bufs=2))
        oout_p = pools.enter_context(tc.tile_pool(name="ooutp", bufs=1))
        pr_ps = pools.enter_context(tc.tile_pool(name="prps", bufs=1, space="PSUM"))
        o_ps = pools.enter_context(tc.tile_pool(name="ops", bufs=1, space="PSUM"))
        st_ps = pools.enter_context(tc.tile_pool(name="stps", bufs=2, space="PSUM"))
        s_ps = pools.enter_context(tc.tile_pool(name="sps", bufs=2, space="PSUM"))
        prC_ps = pr_ps

        _blk_ctr = [0]

        def ln_block(x_dram, pos_dram, blk):
            """DMA a 512-row block of x^T, add pos, compute LN pieces.
            Returns (xh pair of [128,4,512] bf16 half-tiles, m_row [1,512]
            bf16, rrep [128,512] f32 replicated rstd)."""
            _blk_ctr[0] += 1
            ring = nc.sync if _blk_ctr[0] % 2 else nc.scalar  # q0 lands on the empty sync ring
            src = x_dram.rearrange("(c p) n -> p c n", p=128)
            psrc = (pos_dram.rearrange("(c p) n -> p c n", p=128)
                    if pos_dram is not None else None)
            # half-block tiles: DMA -> cast/add -> stats pipeline at finer
            # granularity (first stats matmul fires after half a block)
            xh, xq = [], []
            for hf in range(2):
                xin = xin_p.tile([128, 4, 512], F32, tag="xin", bufs=4)
                ring.dma_start(xin[:], src[:, 4 * hf:4 * hf + 4,
                                           blk * 512:(blk + 1) * 512])
                xbf = xbf_p.tile([128, 4, 512], BF16, tag="xbf", bufs=4)
                if psrc is not None:
                    for qq in range(2):
                        c0 = 4 * hf + 2 * qq
                        xpos = xpos_p.tile([128, 2, 512], F32, tag="xpos")
                        nc.gpsimd.dma_start(
                            xpos[:], psrc[:, c0:c0 + 2, blk * 512:(blk + 1) * 512])
                        nc.vector.tensor_tensor(xbf[:, 2 * qq:2 * qq + 2, :],
                                                xin[:, 2 * qq:2 * qq + 2, :],
                                                xpos[:], op=OP.add)
                else:
                    nc.vector.tensor_copy(xbf[:], xin[:])
                xsq = xsq_p.tile([128, 4, 512], FP8, tag="xsq", bufs=2)
                nc.scalar.activation(xsq[:], xbf[:], AF.Square)
                xh.append(xbf)
                xq.append(xsq)

            p_sum = st_ps.tile([128, 512], F32, tag="stats")
            for c in range(8):
                nc.tensor.matmul(p_sum[:], ones_t[:], xh[c // 4][:, c % 4, :],
                                 start=(c == 0), stop=(c == 7))
            p_sq = st_ps.tile([128, 512], F32, tag="stats")
            for c in range(4):
                nc.tensor.matmul(p_sq[:], ones8[:], xq[c // 2][:, 2 * (c % 2):2 * (c % 2) + 2, :],
                                 start=(c == 0), stop=(c == 3), perf_mode=DR)

            s_sum = sm_p.tile([128, 512], F32, tag="s_sum")
            nc.vector.tensor_copy(s_sum[:], p_sum[:])
            m_row = mrow_p.tile([1, 512], BF16, tag="m_row")
            nc.vector.tensor_scalar(m_row[:], s_sum[0:1, :], 1.0 / D, None, OP.mult)
            msq = sm_p.tile([128, 512], F32, tag="msq", bufs=1)
            nc.vector.scalar_tensor_tensor(msq[:], s_sum[:], 1.0 / D, s_sum[:],
                                           OP.mult, OP.mult)
            v1024 = msq  # D*var overwrites the mean-square scratch in place
            nc.vector.tensor_tensor(v1024[:], p_sq[:], msq[:], op=OP.subtract)
            lnv = sm_p.tile([128, 512], F32, tag="lnv", bufs=1)
            nc.scalar.activation(lnv[:], v1024[:], AF.Ln, bias=eps_t[:],
                                 scale=1.0 / D)
            rrep = rrep_p.tile([128, 512], F32, tag="rrep")
            nc.scalar.activation(rrep[:], lnv[:], AF.Exp, scale=-0.5)
            return xh, m_row, rrep

        def proj_T(xbf, m_row, rrep, w_t, w_l, dst, blk, bw):
            """Transposed projection: dst[:, d, blk*512:...] = (W^T x + aug) * r."""
            for d in range(4):
                pp = pr_ps.tile([128, 512], F32, tag="proj")
                for c in range(8):
                    nc.tensor.matmul(pp[:], w_t[:, c, d * 128:(d + 1) * 128],
                                     xbf[c // 4][:, c % 4, :], start=(c == 0),
                                     stop=False)
                nc.tensor.matmul(pp[:], w_l[:, d * 128:(d + 1) * 128], m_row[:],
                                 start=False, stop=True)
                if bw is not None:
                    nc.vector.scalar_tensor_tensor(
                        dst[:, d, blk * 512:(blk + 1) * 512], pp[:], bw[:, d:d + 1],
                        rrep[:], OP.add, OP.mult)
                else:
                    nc.vector.tensor_tensor(
                        dst[:, d, blk * 512:(blk + 1) * 512], pp[:], rrep[:],
                        op=OP.mult)

        def proj_V(xbf, m_row, rrep, blk):
            """Natural-orientation V projection into vh (65-wide head groups,
            ones column at offset 0 of each group preserved)."""
            for ss in range(4):
                s = blk * 4 + ss
                pv = pr_ps.tile([128, 512], F32, tag="proj")
                for c in range(8):
                    nc.tensor.matmul(pv[:], xbf[c // 4][:, c % 4, ss * 128:(ss + 1) * 128],
                                     wv_t[:, c, :], start=(c == 0), stop=False)
                nc.tensor.matmul(pv[:], m_row[:, ss * 128:(ss + 1) * 128], wv_l[:],
                                 start=False, stop=True)
                # rstd as a per-partition column: tiny DMA transposes the
                # replicated-rstd row [1,128] into a column [128,1]
                rv = rv_p.tile([128, 1], F32, tag="rv")
                nc.scalar.dma_start(rv[:], rrep[0:1, ss * 128:(ss + 1) * 128])
                dst = vh[:, s, :].rearrange("p (h u) -> p h u", u=65)[:, :, 0:64]
                nc.vector.tensor_scalar(
                    dst, pv[:].rearrange("p (h u) -> p h u", u=64), rv[:], None,
                    OP.mult)

        def chain_k(blk):
            xbf, m_row, rrep = ln_block(d_kT, d_kposT, blk)
            proj_T(xbf, m_row, rrep, wk_t, wk_l, khT, blk, None)

        def chain_v(blk):
            xbf, m_row, rrep = ln_block(d_vT, None, blk)
            proj_V(xbf, m_row, rrep, blk)

        # ---- attention (key-halves of 8 kb-blocks, S-groups of 2) -------
        osb1 = {}  # (head, qt) -> unnormalized first-half O, bf16

        def attn_half(head, qt, half):
            hp, hsub = head >> 1, head & 1
            prow = slice(hsub * 64, hsub * 64 + 64)
            O = o_ps.tile([65, 512], F32, tag="O")
            for g in range(4):
                kb0 = half * 8 + g * 2
                S = s_ps.tile([128, 2, 512], F32, tag="S")
                for j in range(2):
                    nc.tensor.matmul(
                        S[:, j, :],
                        khT[prow, hp, (kb0 + j) * 128:(kb0 + j + 1) * 128],
                        qhT[prow, hp, qt * 512:(qt + 1) * 512],
                        start=True, stop=True)
                P = p_sb.tile([128, 2, 512], BF16, tag="P")
                nc.scalar.activation(P[:], S[:], AF.Exp)
                for j in range(2):
                    nc.tensor.matmul(
                        O[:], vh[:, kb0 + j, head * 65:head * 65 + 65], P[:, j, :],
                        start=(g == 0 and j == 0), stop=(g == 3 and j == 1))
            return O

        EPG = 4
        pend = []
        sums_g = None

        def attn_h1(head, qt):
            O = attn_half(head, qt, 0)
            o1 = ep_sb.tile([65, 512], BF16, tag="osb1", bufs=16)
            nc.vector.tensor_copy(o1[:], O[:])
            osb1[(head, qt)] = o1

        def attn_h2(head, qt):
            nonlocal sums_g
            O = attn_half(head, qt, 1)
            ot = ep_sb.tile([65, 512], F32, tag="osbt", bufs=EPG + 1)
            nc.vector.tensor_tensor(ot[:], O[:], osb1[(head, qt)][:], op=OP.add)
            if sums_g is None:
                sums_g = ep_sb.tile([EPG, 512], F32, tag="sums_g")
            nc.scalar.dma_start(sums_g[len(pend):len(pend) + 1, :], ot[64:65, :])
            pend.append((ot, head, qt))
            if len(pend) == EPG:
                flush_epilogue()

        def flush_epilogue():
            nonlocal pend, sums_g
            if not pend:
                return
            n = len(pend)
            rinv_g = ep_sb.tile([EPG, 512], F32, tag="rinv_g")
            nc.vector.reciprocal(rinv_g[0:n, :], sums_g[0:n, :])
            for i, (ot, head, qt) in enumerate(pend):
                hp, hsub = head >> 1, head & 1
                prow = slice(hsub * 64, hsub * 64 + 64)
                rr1 = ep_sb.tile([1, 512], F32, tag="rr1", bufs=1)
                nc.sync.dma_start(rr1[:], rinv_g[i:i + 1, :])
                rr64 = ep_sb.tile([64, 512], F32, tag="rr64")
                nc.gpsimd.partition_broadcast(rr64[:], rr1[:])
                tmp = ep_sb.tile([64, 512], BF16, tag="tmp")
                nc.vector.tensor_tensor(tmp[:], ot[0:64, :], rr64[:], op=OP.mult)
                # scalar ring: its DMA-completion semaphore advances with the
                # frequent sums_g DMAs, so over-conservative reader thresholds
                # (lazy sem rotation) are satisfied promptly
                nc.scalar.dma_start(aout_t[qt][prow, hp, :], tmp[:])
            pend = []
            sums_g = None

        def oproj_qb(qb):
            qt, qc = qb // 4, qb % 4
            osb = oout_p.tile([128, D], F32, tag="osb")
            for half in range(2):
                po = prC_ps.tile([128, 512], F32, tag="proj")
                for hp in range(4):
                    nc.tensor.matmul(po[:], aout_t[qt][:, hp, qc * 128:(qc + 1) * 128],
                                     wp_t[:, hp, half * 512:(half + 1) * 512],
                                     start=(hp == 0), stop=(hp == 3))
                nc.vector.tensor_copy(osb[:, half * 512:(half + 1) * 512], po[:])
            nc.sync.dma_start(d_out[qb * 128:(qb + 1) * 128, :], osb[:])

        # ================= emission =====================================
        # phase A: weights go first on the idle gpsimd ring; v0 (no pos
        # dependency) leads so the PE starts as early as possible
        load_weights()
        chain_v(0)
        lnq0 = ln_block(d_qT, d_qposT, 0)
        proj_T(*lnq0, wq_t, wq_l, qhT, 0, bqw_t)
        chain_k(0)
        lnq1 = ln_block(d_qT, d_qposT, 1)
        proj_T(*lnq1, wq_t, wq_l, qhT, 1, bqw_t)
        chain_k(1); chain_v(1)
        load_wp()
        # attention on the first key-half starts as soon as kb 0-7 exist;
        # the second key-half projections then run underneath its exp stream
        for head in range(NHC):
            for qt in range(2):
                attn_h1(head, qt)
        chain_k(2); chain_v(2); chain_k(3); chain_v(3)
        # phase C: attn half 2 + epilogue; qt0's output projection is
        # interleaved into qt1's attention stream (before qt1's first
        # epilogue flush, so its aout deps are already satisfied)
        for head in range(NHC):
            attn_h2(head, 0)
        flush_epilogue()
        for head in range(NHC):
            attn_h2(head, 1)
            if head == 0:
                for qb in range(4):
                    oproj_qb(qb)
        flush_epilogue()
        for qb in range(4, 8):
            oproj_qb(qb)

    nc.compile()
    return nc


_GRAPH_CACHE = {}


def _graph(has_bqw: bool):
    if has_bqw not in _GRAPH_CACHE:
        _GRAPH_CACHE[has_bqw] = _build_graph(has_bqw)
    return _GRAPH_CACHE[has_bqw]


def kernel(q, k, v, qpos, kpos, gq, bq, gk, bk, gv, bv, Wq, Wk, Wv, Wp, bp):
    f32 = lambda x: np.asarray(x, np.float32)
    q, k, v, qpos, kpos = map(f32, (q, k, v, qpos, kpos))
    gq, bq, gk, bk, gv, bv, Wq, Wk, Wv, Wp, bp = map(
        f32, (gq, bq, gk, bk, gv, bv, Wq, Wk, Wv, Wp, bp))

    Wq_eff = (gq[:, None] * Wq) * SCALE
    Wk_eff = gk[:, None] * Wk
    Wv_eff = gv[:, None] * Wv
    bqw_full = bq @ Wq_eff                      # must be on device if nonzero
    has_bqw = bool(np.any(bqw_full != 0.0))
    extra = (bv @ Wv) @ Wp + bp                 # exact host epilogue

    bf = ml_dtypes.bfloat16

    def aug(w):  # [1024, 512] -> [1025, 512] bf16
        return np.concatenate([w, -w.sum(0, keepdims=True)]).astype(bf)

    whh = []
    for hh in range(2):
        ds = slice(hh * DOUT, (hh + 1) * DOUT)
        whh.append(dict(
            wq=aug(Wq_eff[:, ds]), wk=aug(Wk_eff[:, ds]), wv=aug(Wv_eff[:, ds]),
            wp=Wp[ds, :].astype(bf),
            bqw=np.ascontiguousarray(bqw_full[ds].reshape(4, 128)),
        ))

    kT = [np.ascontiguousarray(k[b].T) for b in range(B)]
    kposT = [np.ascontiguousarray(kpos[b].T) for b in range(B)]
    vT = [np.ascontiguousarray(v[b].T) for b in range(B)]
    qT = [np.ascontiguousarray(q[b].T) for b in range(B)]
    qposT = [np.ascontiguousarray(qpos[b].T) for b in range(B)]

    in_maps = []
    for cid in range(NCORE):
        b, hh, qh = cid >> 2, (cid >> 1) & 1, cid & 1
        qs = slice(qh * NQC, (qh + 1) * NQC)
        m = dict(
            qT=np.ascontiguousarray(qT[b][:, qs]),
            qposT=np.ascontiguousarray(qposT[b][:, qs]),
            kT=kT[b], kposT=kposT[b], vT=vT[b],
            **{kk: vv for kk, vv in whh[hh].items()})
        if not has_bqw:
            m.pop("bqw")
        in_maps.append(m)

    nc = _graph(has_bqw)
    trace = bool(int(os.environ.get("BASS_KERNEL_TRACE", "0")))
    res = run_bass_kernel_spmd(nc, in_maps, core_ids=list(range(NCORE)),
                               trace=trace)
    LAST_RESULT["exec_time_ns"] = res.exec_time_ns
    LAST_RESULT["trace"] = res.instructions_and_trace

    out = np.zeros((B, NQ, D), np.float32)
    for cid in range(NCORE):
        b, hh, qh = cid >> 2, (cid >> 1) & 1, cid & 1
        out[b, qh * NQC:(qh + 1) * NQC, :] += res.results[cid]["out"]
    out += extra[None, None, :]
    return out



# revision 33
# speedup vs baseline: 1.0429x; 1.0429x over previous
"""Pre-LN multi-head attention block on 8 Trainium2 NeuronCores (Bass/Tile).

Reference computation (shapes hardcoded):
    qh = LN(q + qpos) @ Wq ; kh = LN(k + kpos) @ Wk ; vh = LN(v) @ Wv
    out = softmax(qh kh^T / 8) vh @ Wp + bp          (B=2, N=2048, D=1024, H=16)

Sharding (no collectives): 8 cores = (batch b, head-half hh, q-half qh).
Each core computes 8 heads x 1024 q-rows against all 2048 keys and a
partial output projection; the host sums the two head-half partials.

Device algorithm per core (all matmuls bf16 with f32 PSUM accumulation):
  - Host pre-transposes activations (layout only): x^T [1024, rows] f32.
  - LN is folded into the projections: mean-subtraction via an augmented
    1025th weight row (-colsum(W)) against a mean-row appended to x, and
    the rstd multiply rides the PSUM->SBUF copy pass. Stats (sum, sum of
    squares) come from ones-matmuls on the tensor engine; rstd via
    exp(-0.5*ln(var+eps)) on the scalar engine (Rsqrt table is banned).
  - Attention: S^T[keys, q] = khT-block.T @ qhT on PE; P^T = exp(S^T) on
    the scalar engine (no max subtraction needed: |S| < ~3); O^T = V.T@P^T
    with an all-ones column prepended to V so softmax row-sums accumulate
    in PSUM partition 0 of the same matmul; normalize by reciprocal.
  - Output projection from attnout^T chunks; host adds the two partials
    plus the bias epilogue (bv@Wv)@Wp + bp (exact: softmax rows sum to 1;
    k-side bias is softmax-invariant and dropped; q-side bias added on
    device only if nonzero).
"""
import os
import numpy as np
import ml_dtypes

from contextlib import ExitStack
from concourse import bass, bacc, tile, mybir
from concourse.bass_utils import run_bass_kernel_spmd

F32 = mybir.dt.float32
BF16 = mybir.dt.bfloat16
FP8 = mybir.dt.float8e4
AF = mybir.ActivationFunctionType
OP = mybir.AluOpType
DR = mybir.MatmulPerfMode.DoubleRow

B, NQ, NK, D, H = 2, 2048, 2048, 1024, 16
HD = D // H
SCALE = float(HD) ** -0.5
EPS = 1e-5

NCORE = 8
DOUT = 512          # per-core projection width (8 heads)
NQC = 1024          # per-core q rows
NHC = DOUT // HD    # 8 heads per core
NBLK_Q = NQC // 512
NBLK_K = NK // 512

# exec_time_ns of the last run when tracing is enabled (read by test.py)
LAST_RESULT = {}


def _build_graph(has_bqw: bool):
    nc = bacc.Bacc("TRN2", target_bir_lowering=False, debug=False,
                   num_devices=NCORE)

    d_qT = nc.dram_tensor("qT", [D, NQC], F32, kind="ExternalInput").ap()
    d_qposT = nc.dram_tensor("qposT", [D, NQC], F32, kind="ExternalInput").ap()
    d_kT = nc.dram_tensor("kT", [D, NK], F32, kind="ExternalInput").ap()
    d_kposT = nc.dram_tensor("kposT", [D, NK], F32, kind="ExternalInput").ap()
    d_vT = nc.dram_tensor("vT", [D, NK], F32, kind="ExternalInput").ap()
    d_wq = nc.dram_tensor("wq", [D + 1, DOUT], BF16, kind="ExternalInput").ap()
    d_wk = nc.dram_tensor("wk", [D + 1, DOUT], BF16, kind="ExternalInput").ap()
    d_wv = nc.dram_tensor("wv", [D + 1, DOUT], BF16, kind="ExternalInput").ap()
    d_wp = nc.dram_tensor("wp", [DOUT, D], BF16, kind="ExternalInput").ap()
    d_bqw = (nc.dram_tensor("bqw", [4, 128], F32, kind="ExternalInput").ap()
             if has_bqw else None)
    d_out = nc.dram_tensor("out", [NQC, D], F32, kind="ExternalOutput").ap()

    with tile.TileContext(nc) as tc, ExitStack() as es:
        persist = es.enter_context(tc.tile_pool(name="persist", bufs=1))

        # ---- persistent SBUF tensors ------------------------------------
        wq_t = persist.tile([128, 8, DOUT], BF16)
        wk_t = persist.tile([128, 8, DOUT], BF16)
        wv_t = persist.tile([128, 8, DOUT], BF16)
        wq_l = persist.tile([1, DOUT], BF16)
        wk_l = persist.tile([1, DOUT], BF16)
        wv_l = persist.tile([1, DOUT], BF16)
        wp_t = persist.tile([128, 4, D], BF16)
        ones_t = persist.tile([128, 128], BF16)
        ones8 = persist.tile([128, 2, 128], FP8)
        eps_t = persist.tile([128, 1], F32)
        qhT = persist.tile([128, 4, NQC], BF16)            # [512 dout, 1024 q]
        khT = persist.tile([128, 4, NK], BF16)             # [512 dout, 2048 k]
        vh = persist.tile([128, NBLK_K * 4, NHC * 65], BF16)  # per 128-key subblock
        # attnout^T, one tile per q-tile so the qt1 epilogue DMAs never
        # create a (tile-granular) false dep on qt0's output projection
        aout0 = persist.tile([128, 4, 512], BF16)
        aout1 = persist.tile([128, 4, 512], BF16)
        aout_t = [aout0, aout1]
        bqw_t = persist.tile([128, 4], F32) if has_bqw else None

        def load_weights():
            # dedicated (otherwise idle) gpsimd ring so weights never queue
            # behind activation blocks; wv first (v0 projects first)
            for w_t, w_l, d_w in ((wv_t, wv_l, d_wv), (wq_t, wq_l, d_wq),
                                  (wk_t, wk_l, d_wk)):
                nc.gpsimd.dma_start(w_t[:], d_w[0:D, :].rearrange("(c p) n -> p c n", p=128))
                nc.gpsimd.dma_start(w_l[:], d_w[D:D + 1, :])
            if has_bqw:
                nc.gpsimd.dma_start(bqw_t[:], d_bqw.rearrange("d p -> p d"))

        def load_wp():
            # wp isn't needed until phase C; deferring it keeps early HBM
            # bandwidth for the activation blocks
            nc.gpsimd.dma_start(wp_t[:], d_wp.rearrange("(c p) n -> p c n", p=128))

        nc.vector.memset(ones_t[:], 1.0)
        nc.vector.memset(ones8[:], 1.0)
        nc.vector.memset(eps_t[:], EPS)
        # all-ones column at the tail of each 65-wide V group
        nc.vector.memset(vh[:].rearrange("p s (h u) -> p s h u", u=65)[:, :, :, 64:65], 1.0)

        # ---- pools (all live for the whole kernel; PSUM budget = 8 banks:
        #      proj 1 + O 1 + stats 2 + S 2x2 = 8) --------------------------
        pools = es.enter_context(ExitStack())
        xin_p = pools.enter_context(tc.tile_pool(name="xin", bufs=2))
        xpos_p = pools.enter_context(tc.tile_pool(name="xpos", bufs=2))
        xbf_p = pools.enter_context(tc.tile_pool(name="xbf", bufs=2))
        xsq_p = pools.enter_context(tc.tile_pool(name="xsq", bufs=1))
        sm_p = pools.enter_context(tc.tile_pool(name="sm", bufs=2))
        rrep_p = pools.enter_context(tc.tile_pool(name="rrepp", bufs=2))
        mrow_p = pools.enter_context(tc.tile_pool(name="mrowp", bufs=2))
        rv_p = pools.enter_context(tc.tile_pool(name="rvp", bufs=2))
        p_sb = pools.enter_context(tc.tile_pool(name="psb", bufs=2))
        ep_sb = pools.enter_context(tc.tile_pool(name="epsb", bufs=2))
        oout_p = pools.enter_context(tc.tile_pool(name="ooutp", bufs=1))
        pr_ps = pools.enter_context(tc.tile_pool(name="prps", bufs=1, space="PSUM"))
        o_ps = pools.enter_context(tc.tile_pool(name="ops", bufs=1, space="PSUM"))
        st_ps = pools.enter_context(tc.tile_pool(name="stps", bufs=2, space="PSUM"))
        s_ps = pools.enter_context(tc.tile_pool(name="sps", bufs=2, space="PSUM"))
        prC_ps = pr_ps

        _blk_ctr = [0]

        def ln_block(x_dram, pos_dram, blk):
            """DMA a 512-row block of x^T, add pos, compute LN pieces.
            Returns (xh pair of [128,4,512] bf16 half-tiles, m_row [1,512]
            bf16, rrep [128,512] f32 replicated rstd)."""
            _blk_ctr[0] += 1
            ring = nc.sync if _blk_ctr[0] % 2 else nc.scalar  # q0 lands on the empty sync ring
            src = x_dram.rearrange("(c p) n -> p c n", p=128)
            psrc = (pos_dram.rearrange("(c p) n -> p c n", p=128)
                    if pos_dram is not None else None)
            # half-block tiles: DMA -> cast/add -> stats pipeline at finer
            # granularity (first stats matmul fires after half a block)
            xh, xq = [], []
            for hf in range(2):
                xin = xin_p.tile([128, 4, 512], F32, tag="xin", bufs=4)
                ring.dma_start(xin[:], src[:, 4 * hf:4 * hf + 4,
                                           blk * 512:(blk + 1) * 512])
                xbf = xbf_p.tile([128, 4, 512], BF16, tag="xbf", bufs=4)
                if psrc is not None:
                    for qq in range(2):
                        c0 = 4 * hf + 2 * qq
                        xpos = xpos_p.tile([128, 2, 512], F32, tag="xpos")
                        nc.gpsimd.dma_start(
                            xpos[:], psrc[:, c0:c0 + 2, blk * 512:(blk + 1) * 512])
                        nc.vector.tensor_tensor(xbf[:, 2 * qq:2 * qq + 2, :],
                                                xin[:, 2 * qq:2 * qq + 2, :],
                                                xpos[:], op=OP.add)
                else:
                    nc.vector.tensor_copy(xbf[:], xin[:])
                xsq = xsq_p.tile([128, 4, 512], FP8, tag="xsq", bufs=2)
                nc.scalar.activation(xsq[:], xbf[:], AF.Square)
                xh.append(xbf)
                xq.append(xsq)

            p_sum = st_ps.tile([128, 512], F32, tag="stats")
            for c in range(8):
                nc.tensor.matmul(p_sum[:], ones_t[:], xh[c // 4][:, c % 4, :],
                                 start=(c == 0), stop=(c == 7))
            p_sq = st_ps.tile([128, 512], F32, tag="stats")
            for c in range(4):
                nc.tensor.matmul(p_sq[:], ones8[:], xq[c // 2][:, 2 * (c % 2):2 * (c % 2) + 2, :],
                                 start=(c == 0), stop=(c == 3), perf_mode=DR)

            s_sum = sm_p.tile([128, 512], F32, tag="s_sum")
            nc.vector.tensor_copy(s_sum[:], p_sum[:])
            m_row = mrow_p.tile([1, 512], BF16, tag="m_row")
            nc.vector.tensor_scalar(m_row[:], s_sum[0:1, :], 1.0 / D, None, OP.mult)
            msq = sm_p.tile([128, 512], F32, tag="msq", bufs=1)
            nc.vector.scalar_tensor_tensor(msq[:], s_sum[:], 1.0 / D, s_sum[:],
                                           OP.mult, OP.mult)
            v1024 = msq  # D*var overwrites the mean-square scratch in place
            nc.vector.tensor_tensor(v1024[:], p_sq[:], msq[:], op=OP.subtract)
            lnv = sm_p.tile([128, 512], F32, tag="lnv", bufs=1)
            nc.scalar.activation(lnv[:], v1024[:], AF.Ln, bias=eps_t[:],
                                 scale=1.0 / D)
            rrep = rrep_p.tile([128, 512], F32, tag="rrep")
            nc.scalar.activation(rrep[:], lnv[:], AF.Exp, scale=-0.5)
            return xh, m_row, rrep

        def proj_T(xbf, m_row, rrep, w_t, w_l, dst, blk, bw):
            """Transposed projection: dst[:, d, blk*512:...] = (W^T x + aug) * r."""
            for d in range(4):
                pp = pr_ps.tile([128, 512], F32, tag="proj")
                for c in range(8):
                    nc.tensor.matmul(pp[:], w_t[:, c, d * 128:(d + 1) * 128],
                                     xbf[c // 4][:, c % 4, :], start=(c == 0),
                                     stop=False)
                nc.tensor.matmul(pp[:], w_l[:, d * 128:(d + 1) * 128], m_row[:],
                                 start=False, stop=True)
                if bw is not None:
                    nc.vector.scalar_tensor_tensor(
                        dst[:, d, blk * 512:(blk + 1) * 512], pp[:], bw[:, d:d + 1],
                        rrep[:], OP.add, OP.mult)
                else:
                    nc.vector.tensor_tensor(
                        dst[:, d, blk * 512:(blk + 1) * 512], pp[:], rrep[:],
                        op=OP.mult)

        def proj_V(xbf, m_row, rrep, blk):
            """Natural-orientation V projection into vh (65-wide head groups,
            ones column at offset 0 of each group preserved)."""
            for ss in range(4):
                s = blk * 4 + ss
                pv = pr_ps.tile([128, 512], F32, tag="proj")
                for c in range(8):
                    nc.tensor.matmul(pv[:], xbf[c // 4][:, c % 4, ss * 128:(ss + 1) * 128],
                                     wv_t[:, c, :], start=(c == 0), stop=False)
                nc.tensor.matmul(pv[:], m_row[:, ss * 128:(ss + 1) * 128], wv_l[:],
                                 start=False, stop=True)
                # rstd as a per-partition column: tiny DMA transposes the
                # replicated-rstd row [1,128] into a column [128,1]
                rv = rv_p.tile([128, 1], F32, tag="rv")
                nc.scalar.dma_start(rv[:], rrep[0:1, ss * 128:(ss + 1) * 128])
                dst = vh[:, s, :].rearrange("p (h u) -> p h u", u=65)[:, :, 0:64]
                nc.vector.tensor_scalar(
                    dst, pv[:].rearrange("p (h u) -> p h u", u=64), rv[:], None,
                    OP.mult)

        def chain_k(blk):
            xbf, m_row, rrep = ln_block(d_kT, d_kposT, blk)
            proj_T(xbf, m_row, rrep, wk_t, wk_l, khT, blk, None)

        def chain_v(blk):
            xbf, m_row, rrep = ln_block(d_vT, None, blk)
            proj_V(xbf, m_row, rrep, blk)

        # ---- attention (key-halves of 8 kb-blocks, S-groups of 2) -------
        osb1 = {}  # (head, qt) -> unnormalized first-half O, bf16

        def attn_half(head, qt, half):
            hp, hsub = head >> 1, head & 1
            prow = slice(hsub * 64, hsub * 64 + 64)
            O = o_ps.tile([65, 512], F32, tag="O")
            for g in range(4):
                kb0 = half * 8 + g * 2
                S = s_ps.tile([128, 2, 512], F32, tag="S")
                for j in range(2):
                    nc.tensor.matmul(
                        S[:, j, :],
                        khT[prow, hp, (kb0 + j) * 128:(kb0 + j + 1) * 128],
                        qhT[prow, hp, qt * 512:(qt + 1) * 512],
                        start=True, stop=True)
                P = p_sb.tile([128, 2, 512], BF16, tag="P")
                nc.scalar.activation(P[:], S[:], AF.Exp)
                for j in range(2):
                    nc.tensor.matmul(
                        O[:], vh[:, kb0 + j, head * 65:head * 65 + 65], P[:, j, :],
                        start=(g == 0 and j == 0), stop=(g == 3 and j == 1))
            return O

        EPG = 4
        pend = []
        sums_g = None

        def attn_h1(head, qt):
            O = attn_half(head, qt, 0)
            o1 = ep_sb.tile([65, 512], BF16, tag="osb1", bufs=16)
            nc.vector.tensor_copy(o1[:], O[:])
            osb1[(head, qt)] = o1

        def attn_h2(head, qt):
            nonlocal sums_g
            O = attn_half(head, qt, 1)
            ot = ep_sb.tile([65, 512], F32, tag="osbt", bufs=EPG + 1)
            nc.vector.tensor_tensor(ot[:], O[:], osb1[(head, qt)][:], op=OP.add)
            if sums_g is None:
                sums_g = ep_sb.tile([EPG, 512], F32, tag="sums_g")
            nc.scalar.dma_start(sums_g[len(pend):len(pend) + 1, :], ot[64:65, :])
            pend.append((ot, head, qt))
            if len(pend) == EPG:
                flush_epilogue()

        def flush_epilogue():
            nonlocal pend, sums_g
            if not pend:
                return
            n = len(pend)
            rinv_g = ep_sb.tile([EPG, 512], F32, tag="rinv_g")
            nc.vector.reciprocal(rinv_g[0:n, :], sums_g[0:n, :])
            for i, (ot, head, qt) in enumerate(pend):
                hp, hsub = head >> 1, head & 1
                prow = slice(hsub * 64, hsub * 64 + 64)
                rr1 = ep_sb.tile([1, 512], F32, tag="rr1", bufs=1)
                nc.sync.dma_start(rr1[:], rinv_g[i:i + 1, :])
                rr64 = ep_sb.tile([64, 512], F32, tag="rr64")
                nc.gpsimd.partition_broadcast(rr64[:], rr1[:])
                tmp = ep_sb.tile([64, 512], BF16, tag="tmp")
                nc.vector.tensor_tensor(tmp[:], ot[0:64, :], rr64[:], op=OP.mult)
                nc.sync.dma_start(aout_t[qt][prow, hp, :], tmp[:])
            pend = []
            sums_g = None

        def oproj_qb(qb):
            qt, qc = qb // 4, qb % 4
            osb = oout_p.tile([128, D], F32, tag="osb")
            for half in range(2):
                po = prC_ps.tile([128, 512], F32, tag="proj")
                for hp in range(4):
                    nc.tensor.matmul(po[:], aout_t[qt][:, hp, qc * 128:(qc + 1) * 128],
                                     wp_t[:, hp, half * 512:(half + 1) * 512],
                                     start=(hp == 0), stop=(hp == 3))
                nc.vector.tensor_copy(osb[:, half * 512:(half + 1) * 512], po[:])
            nc.sync.dma_start(d_out[qb * 128:(qb + 1) * 128, :], osb[:])

        # ================= emission =====================================
        # phase A: weights go first on the idle gpsimd ring; v0 (no pos
        # dependency) leads so the PE starts as early as possible
        load_weights()
        chain_v(0)
        lnq0 = ln_block(d_qT, d_qposT, 0)
        proj_T(*lnq0, wq_t, wq_l, qhT, 0, bqw_t)
        chain_k(0)
        lnq1 = ln_block(d_qT, d_qposT, 1)
        proj_T(*lnq1, wq_t, wq_l, qhT, 1, bqw_t)
        chain_k(1); chain_v(1)
        load_wp()
        # attention on the first key-half starts as soon as kb 0-7 exist;
        # the second key-half projections then run underneath its exp stream
        for head in range(NHC):
            for qt in range(2):
                attn_h1(head, qt)
        chain_k(2); chain_v(2); chain_k(3); chain_v(3)
        # phase C: attn half 2 + epilogue; qt0's output projection is
        # interleaved into qt1's attention stream (before qt1's first
        # epilogue flush, so its aout deps are already satisfied)
        for head in range(NHC):
            attn_h2(head, 0)
        flush_epilogue()
        for head in range(NHC):
            attn_h2(head, 1)
            if head == 0:
                for qb in range(4):
                    oproj_qb(qb)
        flush_epilogue()
        for qb in range(4, 8):
            oproj_qb(qb)

    nc.compile()
    return nc


_GRAPH_CACHE = {}


def _graph(has_bqw: bool):
    if has_bqw not in _GRAPH_CACHE:
        _GRAPH_CACHE[has_bqw] = _build_graph(has_bqw)
    return _GRAPH_CACHE[has_bqw]


def kernel(q, k, v, qpos, kpos, gq, bq, gk, bk, gv, bv, Wq, Wk, Wv, Wp, bp):
    f32 = lambda x: np.asarray(x, np.float32)
    q, k, v, qpos, kpos = map(f32, (q, k, v, qpos, kpos))
    gq, bq, gk, bk, gv, bv, Wq, Wk, Wv, Wp, bp = map(
        f32, (gq, bq, gk, bk, gv, bv, Wq, Wk, Wv, Wp, bp))

    Wq_eff = (gq[:, None] * Wq) * SCALE
    Wk_eff = gk[:, None] * Wk
    Wv_eff = gv[:, None] * Wv
    bqw_full = bq @ Wq_eff                      # must be on device if nonzero
    has_bqw = bool(np.any(bqw_full != 0.0))
    extra = (bv @ Wv) @ Wp + bp                 # exact host epilogue

    bf = ml_dtypes.bfloat16

    def aug(w):  # [1024, 512] -> [1025, 512] bf16
        return np.concatenate([w, -w.sum(0, keepdims=True)]).astype(bf)

    whh = []
    for hh in range(2):
        ds = slice(hh * DOUT, (hh + 1) * DOUT)
        whh.append(dict(
            wq=aug(Wq_eff[:, ds]), wk=aug(Wk_eff[:, ds]), wv=aug(Wv_eff[:, ds]),
            wp=Wp[ds, :].astype(bf),
            bqw=np.ascontiguousarray(bqw_full[ds].reshape(4, 128)),
        ))

    kT = [np.ascontiguousarray(k[b].T) for b in range(B)]
    kposT = [np.ascontiguousarray(kpos[b].T) for b in range(B)]
    vT = [np.ascontiguousarray(v[b].T) for b in range(B)]
    qT = [np.ascontiguousarray(q[b].T) for b in range(B)]
    qposT = [np.ascontiguousarray(qpos[b].T) for b in range(B)]

    in_maps = []
    for cid in range(NCORE):
        b, hh, qh = cid >> 2, (cid >> 1) & 1, cid & 1
        qs = slice(qh * NQC, (qh + 1) * NQC)
        m = dict(
            qT=np.ascontiguousarray(qT[b][:, qs]),
            qposT=np.ascontiguousarray(qposT[b][:, qs]),
            kT=kT[b], kposT=kposT[b], vT=vT[b],
            **{kk: vv for kk, vv in whh[hh].items()})
        if not has_bqw:
            m.pop("bqw")
        in_maps.append(m)

    nc = _graph(has_bqw)
    trace = bool(int(os.environ.get("BASS_KERNEL_TRACE", "0")))
    res = run_bass_kernel_spmd(nc, in_maps, core_ids=list(range(NCORE)),
                               trace=trace)
    LAST_RESULT["exec_time_ns"] = res.exec_time_ns
    LAST_RESULT["trace"] = res.instructions_and_trace

    out = np.zeros((B, NQ, D), np.float32)
    for cid in range(NCORE):
        b, hh, qh = cid >> 2, (cid >> 1) & 1, cid & 1
        out[b, qh * NQC:(qh + 1) * NQC, :] += res.results[cid]["out"]
    out += extra[None, None, :]
    return out

